# revision 36
# baseline (speedup 1.0000x reference)
"""Trainium2 Bass kernel for BottleneckAttention (patch attention), fp8 edition.

q patches [160, 5120] from z1_hat (non-overlapping 10x4 unfold),
kv patches [5551, 5120] from z2 (overlapping unfold, Hk=91 x Wk=61),
scores = q @ kv.T / 5120, softmax over kv patches, out = attn @ kv,
folded back to [1, 128, 100, 64].

Sharding: contiguous blocks of 12 kv h-rows per core (8 x 12 = 96 >= 91).
Each core owns the 768 flat positions p = h_local*64 + w (w in [0,64));
positions with w >= 61 or h >= 91 are invalid -- their kv rows are zeroed
so they never touch the numerator, and the host subtracts their exactly
recomputed exp contribution from the denominator. Every core computes all
160 q rows; the host combines with an all-gather softmax.

Per-core kernel (raw Bass, explicit semaphores), fp8e4m3 everywhere on
the PE with DoubleRow (K=256) perf mode for the M=128 matmul blocks:
  phase 1: scores as implicit convolution against the SBUF-resident
    z2 slab (zz holds the slab plus a 64-shifted copy so (i, i+1) kernel
    row pairs form clean [128, 2, N] DoubleRow moving operands).
    q rows 0..127 run M=128 DoubleRow; rows 128..159 run as three
    concurrent 32-wide PE column groups (no DoubleRow -- col tiling and
    DoubleRow are mutually exclusive).
  exp on ScalarE with scale=1/5120 and bias=ln(64): e64 = 64*exp(s).
  row-sum denominator (64x) on VectorE; host divides by 64.
  PE transposes of e64 chunks; the ACT psum->sbuf copy applies bias=-64
  so the fp8 fT stores f64 = 64*(e-1) (centered softmax, scaled into
  fp8e4m3's normal range; the host adds the exact sum-of-kv-columns term
  and divides by 64).
  phase 2: partial_out = f64T.T @ kv_shard in fp8 DoubleRow (m0) plus
    three-column-group fp8 (m1), kv resident in SBUF, drained to bf16.
"""

import sys

sys.path.insert(0, "/opt/trn_rl_repo")

import numpy as np
import ml_dtypes

import concourse.bass as bass
import concourse.mybir as mybir

DT = mybir.dt
AF = mybir.ActivationFunctionType
PM = mybir.MatmulPerfMode

# problem geometry (hardcoded from the reference module)
KC, KH, KW = 128, 10, 4
H, W = 100, 64
NH, NW = H // KH, W // KW          # 10, 16
PQ = NH * NW                       # 160 q patches
D = KC * KH * KW                   # 5120
HK, WK = H - KH + 1, W - KW + 1    # 91, 61
NCORES = 8
HPC = 12                           # kv h-rows per core (8*12 = 96 >= 91)
PKC = HPC * W                      # 768 flat positions per core
ZROWS = 2 * HPC                    # 24 z rows staged per core
ZLEN = ZROWS * W                   # 1536
SCALE = 1.0 / D
LN64 = float(np.log(64.0))
F8 = ml_dtypes.float8_e4m3fn

_CACHE = {}


def _build_nc():
    nc = bass.Bass()
    zz_d = nc.declare_dram_parameter("zz", [KC, 2, ZLEN], DT.float8e4, isOutput=False)
    qm1_d = nc.declare_dram_parameter("qm1", [KC, KW, KH, 32], DT.float8e4, isOutput=False)
    idc_d = nc.declare_dram_parameter("idc", [128, 129], DT.float32, isOutput=False)
    qm0_d = nc.declare_dram_parameter("qm0", [KC, KW, KH, 128], DT.float8e4, isOutput=False)
    kv_d = nc.declare_dram_parameter("kv8", [128, 6, D], DT.float8e4, isOutput=False)
    ohi_d = nc.declare_dram_parameter("ohi", [128, D], DT.float8e4, isOutput=True)
    olo_d = nc.declare_dram_parameter("olo", [96, 4, 512], DT.float8e4, isOutput=True)
    den_d = nc.declare_dram_parameter("den", [224, 1], DT.float32, isOutput=True)

    from contextlib import ExitStack

    ctx = ExitStack()
    with ctx:
        zz_sb = ctx.enter_context(nc.sbuf_tensor([KC, 2, ZLEN], DT.float8e4))
        qm1_sb = ctx.enter_context(nc.sbuf_tensor([KC, KW, KH, 32], DT.float8e4))
        qm0_sb = ctx.enter_context(nc.sbuf_tensor([KC, KW, KH, 128], DT.float8e4))
        idc = ctx.enter_context(nc.sbuf_tensor([128, 129], DT.float32))
        kv_sb = ctx.enter_context(nc.sbuf_tensor([128, 6, D], DT.float8e4))
        e_hi = ctx.enter_context(nc.sbuf_tensor([128, PKC], DT.float32))
        e_lo = ctx.enter_context(nc.sbuf_tensor([96, 256], DT.float32))
        fT = ctx.enter_context(nc.sbuf_tensor([128, 6, PQ], DT.float8e4))
        o_hi = ctx.enter_context(nc.sbuf_tensor([128, D], DT.float8e4))
        o_lo = ctx.enter_context(nc.sbuf_tensor([96, 4, 512], DT.float8e4))
        dh_sb = ctx.enter_context(nc.sbuf_tensor([128, 1], DT.float32))
        dl_sb = ctx.enter_context(nc.sbuf_tensor([96, 1], DT.float32))
        scr = ctx.enter_context(nc.sbuf_tensor([128, 8], DT.float32))
        wz = ctx.enter_context(nc.sbuf_tensor([128, 512], DT.float8e4))

        ps_a = ctx.enter_context(nc.psum_tensor("ps_a", [128, 512], DT.float32))
        ps_b = ctx.enter_context(nc.psum_tensor("ps_b", [128, 512], DT.float32))
        ps_m = ctx.enter_context(nc.psum_tensor("ps_m", [128, 512], DT.float32))
        ps_w = ctx.enter_context(nc.psum_tensor("ps_w", [128, 512], DT.float32))
        ps_t = [
            ctx.enter_context(nc.psum_tensor(f"ps_t{i}", [128, 512], DT.float32))
            for i in range(4)
        ]

        s_z = ctx.enter_context(nc.semaphore("s_z"))
        s_z2 = ctx.enter_context(nc.semaphore("s_z2"))
        s_q1 = ctx.enter_context(nc.semaphore("s_q1"))
        s_q0 = ctx.enter_context(nc.semaphore("s_q0"))
        s_i = ctx.enter_context(nc.semaphore("s_i"))
        s_kv = [ctx.enter_context(nc.semaphore(f"s_kv{i}")) for i in range(3)]
        s_p = ctx.enter_context(nc.semaphore("s_p"))
        s_a = ctx.enter_context(nc.semaphore("s_a"))
        s_v = ctx.enter_context(nc.semaphore("s_v"))
        s_o = ctx.enter_context(nc.semaphore("s_o"))
        s_g = ctx.enter_context(nc.semaphore("s_g"))

        # p1 m1 col-groups: gA->ps_m[0:32], gB->ps_t0[32:64], gC->ps_t1[64:96]
        M1B = [ps_m, ps_t[0], ps_t[1]]
        # TR1: two 96-row transposes of e_lo halves; each yields the three
        # fT m1 chunks of its column half (col groups stack in the output)
        TR1_BANK = [ps_t[2], ps_t[3]]
        # TR0 emission order: e_hi chunks [4,5,0,1,2,3] on banks [t0,t1,t2,t3,t0,t1]
        TR0_CHUNK = [4, 5, 0, 1, 2, 3]
        TR0_BANK = [ps_t[0], ps_t[1], ps_t[2], ps_t[3], ps_a, ps_t[0]]
        TR0_SA = [4, 4, 5, 5, 5, 5]       # chunks 4,5 <- exp B; 0..3 <- exp A
        TR0_SV = [0, 0, 4, 7, 0, 8]
        # phase-2 m1 triples (n-tiles 3r..3r+2) on banks (w, b, m)
        RB = [ps_w, ps_b, ps_m]
        TRIP_SA = [4, 7, 9, 11]
        TRIP_SV = [7, 10, 16, 0]
        TRIP_KV = [[0], [1], [1, 2], [2]]

        with nc.Block() as block:

            @block.sync
            def _(sync):
                sync.dma_start(zz_sb[:, 0, :], zz_d[:, 0, :]).then_inc(s_z, 16)
                sync.dma_start(qm1_sb[:, :, :, :], qm1_d[:]).then_inc(s_q1, 16)
                sync.dma_start(zz_sb[:, 1, :], zz_d[:, 1, :]).then_inc(s_z2, 16)
                sync.dma_start(qm0_sb[:, :, :, :], qm0_d[:]).then_inc(s_q0, 16)
                sync.dma_start(idc[:, :], idc_d[:]).then_inc(s_i, 16)
                sync.dma_start(kv_sb[:, :, 0:1536], kv_d[:, :, 0:1536]).then_inc(
                    s_kv[0], 16
                )
                sync.dma_start(kv_sb[:, :, 1536:3584], kv_d[:, :, 1536:3584]).then_inc(
                    s_kv[1], 16
                )
                sync.dma_start(kv_sb[:, :, 3584:5120], kv_d[:, :, 3584:5120]).then_inc(
                    s_kv[2], 16
                )
                sync.wait_ge(s_v, 1)
                sync.dma_start(den_d[128:224, :], dl_sb[:]).then_inc(s_o, 16)
                sync.wait_ge(s_a, 7)
                sync.wait_ge(s_v, 10)
                sync.dma_start(olo_d[:, 0, :], o_lo[:, 0, :]).then_inc(s_o, 16)
                sync.wait_ge(s_v, 15)
                sync.dma_start(den_d[0:128, :], dh_sb[:]).then_inc(s_o, 16)
                sync.wait_ge(s_a, 9)
                sync.wait_ge(s_v, 16)
                sync.dma_start(olo_d[:, 1, :], o_lo[:, 1, :]).then_inc(s_o, 16)
                sync.wait_ge(s_a, 10)
                sync.wait_ge(s_v, 17)
                sync.dma_start(ohi_d[:, 0:1024], o_hi[:, 0:1024]).then_inc(s_o, 16)
                sync.wait_ge(s_a, 12)
                sync.wait_ge(s_v, 18)
                sync.dma_start(olo_d[:, 2, :], o_lo[:, 2, :]).then_inc(s_o, 16)
                sync.wait_ge(s_a, 13)
                sync.wait_ge(s_v, 19)
                sync.dma_start(ohi_d[:, 1024:2048], o_hi[:, 1024:2048]).then_inc(
                    s_o, 16
                )
                sync.wait_ge(s_a, 14)
                sync.wait_ge(s_v, 20)
                sync.dma_start(ohi_d[:, 2048:3072], o_hi[:, 2048:3072]).then_inc(
                    s_o, 16
                )
                sync.wait_ge(s_a, 15)
                sync.wait_ge(s_v, 21)
                sync.dma_start(ohi_d[:, 3072:4096], o_hi[:, 3072:4096]).then_inc(
                    s_o, 16
                )
                sync.wait_ge(s_o, 176)

            @block.tensor
            def _(pe):
                # HAM warmup on the DVE-memset tile until the z/q DMAs land
                pe.wait_ge(s_g, 1)
                for w in range(6):
                    nc.tensor.matmul(
                        ps_w[0:128, 0:512],
                        wz[:, 0:128],
                        wz[:, 0:512],
                        start=(w == 0),
                        stop=(w == 5),
                    )
                pe.wait_ge(s_z, 16)
                pe.wait_ge(s_q1, 16)
                # phase 1 m1 (q rows 128..159): 3 concurrent 32-col groups
                mfin = [None, None, None]
                for i in range(KH):
                    for j in range(KW):
                        st = i == 0 and j == 0
                        sp = i == KH - 1 and j == KW - 1
                        off = i * W + j
                        for g in range(3):
                            mfin[g] = nc.tensor.matmul(
                                M1B[g][32 * g : 32 * g + 32, 0:256],
                                qm1_sb[:, j, i, :],
                                zz_sb[:, 0, off + 256 * g : off + 256 * g + 256],
                                start=st,
                                stop=sp,
                            )
                for g in range(3):
                    mfin[g].then_inc(s_p, 1)  # s_p = 1, 2, 3
                pe.wait_ge(s_q0, 16)
                pe.wait_ge(s_z2, 16)
                # phase 1 m0 chain B (pos 512:768), DoubleRow (i, i+1) pairs
                for j in range(KW):
                    for ip in range(5):
                        st = j == 0 and ip == 0
                        sp = j == KW - 1 and ip == 4
                        off = (2 * ip) * W + j
                        mm = nc.tensor.matmul(
                            ps_b[0:128, 0:256],
                            qm0_sb[:, j, 2 * ip : 2 * ip + 2, :],
                            zz_sb[:, :, off + 512 : off + 768],
                            start=st,
                            stop=sp,
                            perf_mode=PM.DoubleRow,
                        )
                mm.then_inc(s_p, 1)  # s_p = 4
                # TR1: transpose e_lo (m1) halves -> banks t2/t3
                pe.wait_ge(s_i, 16)
                pe.wait_ge(s_a, 3)
                for k in range(2):
                    nc.tensor.matmul(
                        TR1_BANK[k][0:128, 0:96],
                        e_lo[0:96, 128 * k : 128 * k + 128],
                        idc[0:96, 0:96],
                        is_transpose=True,
                        start=True,
                        stop=True,
                    ).then_inc(s_p, 1)  # s_p = 5, 6
                # phase 1 m0 chain A (pos 0:512)
                for j in range(KW):
                    for ip in range(5):
                        st = j == 0 and ip == 0
                        sp = j == KW - 1 and ip == 4
                        off = (2 * ip) * W + j
                        mm = nc.tensor.matmul(
                            ps_a[0:128, 0:512],
                            qm0_sb[:, j, 2 * ip : 2 * ip + 2, :],
                            zz_sb[:, :, off : off + 512],
                            start=st,
                            stop=sp,
                            perf_mode=PM.DoubleRow,
                        )
                mm.then_inc(s_p, 1)  # s_p = 7

                def p2triple(r):
                    pe.wait_ge(s_a, TRIP_SA[r])
                    if TRIP_SV[r]:
                        pe.wait_ge(s_v, TRIP_SV[r])
                    for pc in TRIP_KV[r]:
                        pe.wait_ge(s_kv[pc], 16)
                    ng = 3 if r < 3 else 1
                    mf = [None] * ng
                    for t6 in range(6):
                        st, sp = t6 == 0, t6 == 5
                        for g in range(ng):
                            mf[g] = nc.tensor.matmul(
                                RB[g][32 * g : 32 * g + 32, 0:512],
                                fT[:, t6, 128:160],
                                kv_sb[:, t6, (3 * r + g) * 512 : (3 * r + g + 1) * 512],
                                start=st,
                                stop=sp,
                            )
                    for g in range(ng):
                        mf[g].then_inc(s_p, 1)

                # r0 runs in the exp-A latency shadow right after chain A
                p2triple(0)  # s_p = 12, 13, 14
                # TR0: transpose e_hi chunks, exp-B-dependent chunks first
                for k in range(6):
                    c = TR0_CHUNK[k]
                    pe.wait_ge(s_a, TR0_SA[k])
                    if TR0_SV[k]:
                        pe.wait_ge(s_v, TR0_SV[k])
                    nc.tensor.matmul(
                        TR0_BANK[k][0:128, 0:128],
                        e_hi[:, c * 128 : (c + 1) * 128],
                        idc[0:128, 0:128],
                        is_transpose=True,
                        start=True,
                        stop=True,
                    ).then_inc(s_p, 1)  # s_p = 11..16

                def p2pair(k, banks, sa, sv, kvs, sv2=0):
                    if sa:
                        pe.wait_ge(s_a, sa)
                    pe.wait_ge(s_v, sv)
                    for pc in kvs:
                        pe.wait_ge(s_kv[pc], 16)
                    bA, bB = ps_t[banks[0]], ps_t[banks[1]]
                    for tp in range(3):
                        if tp == 1 and sv2:
                            pe.wait_ge(s_v, sv2)
                        st, sp = tp == 0, tp == 2
                        mA = nc.tensor.matmul(
                            bA[0:128, 0:512],
                            fT[:, 2 * tp : 2 * tp + 2, 0:128],
                            kv_sb[
                                :, 2 * tp : 2 * tp + 2, (2 * k) * 512 : (2 * k + 1) * 512
                            ],
                            start=st,
                            stop=sp,
                            perf_mode=PM.DoubleRow,
                        )
                        mB = nc.tensor.matmul(
                            bB[0:128, 0:512],
                            fT[:, 2 * tp : 2 * tp + 2, 0:128],
                            kv_sb[
                                :,
                                2 * tp : 2 * tp + 2,
                                (2 * k + 1) * 512 : (2 * k + 2) * 512,
                            ],
                            start=st,
                            stop=sp,
                            perf_mode=PM.DoubleRow,
                        )
                    mA.then_inc(s_p, 1)
                    mB.then_inc(s_p, 1)

                p2triple(1)                       # s_p = 21, 22, 23
                p2pair(0, (2, 3), 0, 12, [0], sv2=14)  # s_p = 20, 21
                p2triple(2)                       # s_p = 26, 27, 28
                p2pair(1, (0, 1), 0, 14, [0, 1])  # s_p = 25, 26
                p2pair(2, (2, 3), 10, 17, [1])    # s_p = 27, 28
                p2pair(3, (0, 1), 13, 19, [1, 2])  # s_p = 29, 30
                p2triple(3)                       # s_p = 35
                p2pair(4, (2, 3), 14, 20, [2])    # s_p = 32, 33

            @block.scalar
            def _(act):
                # warm the exp table set early (reads the memset tile)
                act.wait_ge(s_g, 1)
                nc.scalar.activation(
                    scr[:, :], wz[:, 0:8], AF.Exp, bias=0.0, scale=1.0
                )
                act.wait_ge(s_i, 16)
                # e64 = 64 * exp(s * SCALE); bias AP holds ln(64)
                for g in range(3):
                    act.wait_ge(s_p, 1 + g)
                    nc.scalar.activation(
                        e_lo[32 * g : 32 * g + 32, 0:256],
                        M1B[g][32 * g : 32 * g + 32, 0:256],
                        AF.Exp,
                        bias=idc[32 * g : 32 * g + 32, 128:129],
                        scale=SCALE,
                    ).then_inc(s_a, 1)  # 1, 2, 3
                act.wait_ge(s_p, 4)
                nc.scalar.activation(
                    e_hi[:, 512:768], ps_b[0:128, 0:256], AF.Exp,
                    bias=idc[:, 128:129], scale=SCALE,
                ).then_inc(s_a, 1)  # 4 (exp B)
                act.wait_ge(s_p, 7)
                nc.scalar.activation(
                    e_hi[:, 0:512], ps_a[0:128, 0:512], AF.Exp,
                    bias=idc[:, 128:129], scale=SCALE,
                ).then_inc(s_a, 1)  # 5 (exp A)

                def m1drain(gi, r, spv):
                    act.wait_ge(s_p, spv)
                    nc.scalar.activation(
                        o_lo[32 * gi : 32 * gi + 32, r, :],
                        RB[gi][32 * gi : 32 * gi + 32, 0:512],
                        AF.Copy,
                    ).then_inc(s_a, 1)

                def ntdrain_a(g, spv, bank):
                    act.wait_ge(s_p, spv)
                    nc.scalar.activation(
                        o_hi[:, g * 512 : (g + 1) * 512],
                        bank[0:128, 0:512],
                        AF.Copy,
                    ).then_inc(s_a, 1)

                m1drain(0, 0, 8)             # 6
                m1drain(1, 0, 9)             # 7
                m1drain(0, 1, 17)            # 8
                m1drain(1, 1, 18)            # 9
                ntdrain_a(1, 21, ps_t[3])    # 10
                m1drain(0, 2, 22)            # 11
                m1drain(1, 2, 23)            # 12
                ntdrain_a(3, 26, ps_t[1])    # 13
                ntdrain_a(5, 28, ps_t[3])    # 14
                ntdrain_a(7, 30, ps_t[1])    # 15
                m1drain(0, 3, 31)            # 16
                act.wait_ge(s_a, 16)
                nc.scalar.dma_start(olo_d[0:32, 3, :], o_lo[0:32, 3, :]).then_inc(
                    s_o, 16
                )
                ntdrain_a(9, 33, ps_t[3])    # 17
                act.wait_ge(s_a, 17)
                act.wait_ge(s_v, 22)
                nc.scalar.dma_start(
                    ohi_d[:, 4096:5120], o_hi[:, 4096:5120]
                ).then_inc(s_o, 16)

            @block.vector
            def _(dve):
                nc.vector.memset(wz[:, :], 0.0).then_inc(s_g, 1)
                dve.wait_ge(s_a, 3)
                nc.vector.reduce_sum(
                    dl_sb[:], e_lo[:, :], axis=mybir.AxisListType.X
                ).then_inc(s_v, 1)  # 1
                # fT m1 copies: f64 = e64T - 64, cast to fp8
                for k in range(2):
                    dve.wait_ge(s_p, 5 + k)
                    for g in range(3):
                        nc.vector.tensor_scalar_sub(
                            fT[:, 2 * g + k, 128:160],
                            TR1_BANK[k][0:128, 32 * g : 32 * g + 32],
                            64.0,
                        ).then_inc(s_v, 1)  # 2..7

                def gc_drain(r, sp_val):
                    dve.wait_ge(s_p, sp_val)
                    nc.vector.tensor_copy(
                        o_lo[64:96, r, :], ps_m[64:96, 0:512]
                    ).then_inc(s_v, 1)

                def nt_drain(g, sp_val, bank):
                    dve.wait_ge(s_p, sp_val)
                    nc.vector.tensor_copy(
                        o_hi[:, g * 512 : (g + 1) * 512], bank[0:128, 0:512]
                    ).then_inc(s_v, 1)

                dve.wait_ge(s_p, 12)
                for k in range(2):
                    nc.vector.tensor_scalar_sub(
                        fT[:, TR0_CHUNK[k], 0:128], TR0_BANK[k][0:128, 0:128], 64.0
                    ).then_inc(s_v, 1)  # 8, 9
                gc_drain(0, 10)            # 10
                dve.wait_ge(s_p, 14)
                for k in range(2, 4):
                    nc.vector.tensor_scalar_sub(
                        fT[:, TR0_CHUNK[k], 0:128], TR0_BANK[k][0:128, 0:128], 64.0
                    ).then_inc(s_v, 1)  # 11, 12
                dve.wait_ge(s_p, 16)
                for k in range(4, 6):
                    nc.vector.tensor_scalar_sub(
                        fT[:, TR0_CHUNK[k], 0:128], TR0_BANK[k][0:128, 0:128], 64.0
                    ).then_inc(s_v, 1)  # 13, 14
                dve.wait_ge(s_a, 5)
                nc.vector.reduce_sum(
                    dh_sb[:], e_hi[:, :], axis=mybir.AxisListType.X
                ).then_inc(s_v, 1)  # 15
                gc_drain(1, 19)            # 16
                nt_drain(0, 20, ps_t[2])   # 17
                gc_drain(2, 24)            # 18
                nt_drain(2, 25, ps_t[0])   # 19
                nt_drain(4, 27, ps_t[2])   # 20
                nt_drain(6, 29, ps_t[0])   # 21
                nt_drain(8, 32, ps_t[2])   # 22

    return nc


def _host_prep(z1_hat, z2):
    z1 = np.asarray(z1_hat, dtype=np.float32)[0]  # [128, 100, 64]
    z2a = np.asarray(z2, dtype=np.float32)[0]

    # q patches [160, 5120]; device layout q8 [128, j, i, 160]
    q = z1.reshape(KC, NH, KH, NW, KW).transpose(1, 3, 0, 2, 4).reshape(PQ, D)
    q4 = q.reshape(PQ, KC, KH, KW)
    q8 = q4.transpose(1, 3, 2, 0).astype(F8)  # [c, j, i, p]
    qm0 = np.ascontiguousarray(q8[:, :, :, 0:128])
    qm1 = np.ascontiguousarray(q8[:, :, :, 128:160])

    # padded z2: rows 100..111 zero
    z_pad = np.zeros((KC, 112, W), dtype=np.float32)
    z_pad[:, :H] = z2a
    z8_pad = z_pad.astype(F8)

    # sliding kv patches from padded z2 (original fp32 values, cast per-row)
    sw = np.lib.stride_tricks.sliding_window_view(z_pad, (KH, KW), axis=(1, 2))

    q64 = q.astype(np.float64)
    ij_off = (np.arange(KH)[:, None] * W + np.arange(KW)[None, :]).reshape(-1)  # [40]

    idc = np.zeros((128, 129), dtype=np.float32)
    idc[:, 0:128] = np.eye(128, dtype=np.float32)
    idc[:, 128] = LN64

    in_maps = []
    corrs = []
    for core in range(NCORES):
        h0 = HPC * core
        slab8 = z8_pad[:, h0 : h0 + ZROWS, :].reshape(KC, ZLEN)
        zz = np.zeros((KC, 2, ZLEN), dtype=F8)
        zz[:, 0, :] = slab8
        zz[:, 1, 0 : ZLEN - W] = slab8[:, W:]
        # kv rows indexed by flat position p = h_local*64 + w
        kvp = np.zeros((PKC, D), dtype=F8)
        hh = np.arange(PKC) // W
        ww = np.arange(PKC) % W
        real = (ww < WK) & (h0 + hh < HK)
        ridx = np.nonzero(real)[0]
        kvp[ridx] = (
            sw[:, h0 + hh[ridx], ww[ridx]].transpose(1, 0, 2, 3).reshape(-1, D)
        ).astype(F8)
        kv8 = np.ascontiguousarray(kvp.reshape(6, 128, D).transpose(1, 0, 2))
        in_maps.append(
            {"zz": zz, "qm0": qm0, "qm1": qm1, "kv8": kv8, "idc": idc}
        )
        # denominator correction: computed-but-invalid stream positions,
        # recomputed exactly (fp64) from the original values.
        bad = np.nonzero(~real)[0]
        zf = z_pad[:, h0 : h0 + ZROWS, :].reshape(KC, ZLEN).astype(np.float64)
        win = zf[:, bad[:, None] + ij_off[None, :]]  # [128, nb, 40]
        patches = win.transpose(1, 0, 2).reshape(len(bad), D)
        s_bad = q64 @ patches.T  # [160, nb]
        corrs.append(np.exp(s_bad * SCALE).sum(axis=1))

    corr = np.sum(corrs, axis=0)
    swr = sw[:, :HK, :WK]
    colsum = swr.astype(np.float64).sum(axis=(1, 2)).reshape(D)  # [5120]
    return in_maps, corr, colsum


def kernel(z1_hat, z2):
    from concourse.bass_utils import run_bass_kernel_spmd

    in_maps, corr, colsum = _host_prep(z1_hat, z2)
    if "nc" not in _CACHE:
        _CACHE["nc"] = _build_nc()
    nc = _CACHE["nc"]
    res = run_bass_kernel_spmd(nc, in_maps, list(range(NCORES)))
    num = np.broadcast_to(colsum, (PQ, D)).astype(np.float64).copy()
    den = -corr
    for r in res.results:
        ohi = r["ohi"].astype(np.float64)  # [128, 5120] = 64 * partial m0
        olo = r["olo"].astype(np.float64).reshape(96, 4, 512)
        part = np.zeros((PQ, D))
        part[0:128] = ohi
        for nt in range(10):
            rr, g = nt // 3, nt % 3
            part[128:160, nt * 512 : (nt + 1) * 512] = olo[32 * g : 32 * g + 32, rr]
        num += part / 64.0
        dv = r["den"].astype(np.float64)[:, 0] / 64.0
        den = den + np.concatenate(
            [dv[0:128], dv[128:160] + dv[160:192] + dv[192:224]]
        )
    out = (num / den[:, None]).astype(np.float32)
    # fold patches back: [160, 5120] -> [1, 128, 100, 64]
    out = out.reshape(NH, NW, KC, KH, KW).transpose(2, 0, 3, 1, 4)
    return np.ascontiguousarray(out.reshape(1, KC, H, W))


# revision 37
# speedup vs baseline: 1.0140x; 1.0140x over previous
"""Trainium2 Bass kernel for BottleneckAttention (patch attention), fp8 edition.

q patches [160, 5120] from z1_hat (non-overlapping 10x4 unfold),
kv patches [5551, 5120] from z2 (overlapping unfold, Hk=91 x Wk=61),
scores = q @ kv.T / 5120, softmax over kv patches, out = attn @ kv,
folded back to [1, 128, 100, 64].

Sharding: contiguous blocks of 12 kv h-rows per core (8 x 12 = 96 >= 91).
Each core owns the 768 flat positions p = h_local*64 + w (w in [0,64));
positions with w >= 61 or h >= 91 are invalid -- their kv rows are zeroed
so they never touch the numerator, and the host subtracts their exactly
recomputed exp contribution from the denominator. Every core computes all
160 q rows; the host combines with an all-gather softmax.

Per-core kernel (raw Bass, explicit semaphores), fp8e4m3 everywhere on
the PE with DoubleRow (K=256) perf mode for the M=128 matmul blocks:
  phase 1: scores as implicit convolution against the SBUF-resident
    z2 slab (zz holds the slab plus a 64-shifted copy so (i, i+1) kernel
    row pairs form clean [128, 2, N] DoubleRow moving operands).
    q rows 0..127 run M=128 DoubleRow; rows 128..159 run as three
    concurrent 32-wide PE column groups (no DoubleRow -- col tiling and
    DoubleRow are mutually exclusive).
  exp on ScalarE with scale=1/5120 and bias=ln(64): e64 = 64*exp(s).
  row-sum denominator (64x) on VectorE; host divides by 64.
  PE transposes of e64 chunks; the ACT psum->sbuf copy applies bias=-64
  so the fp8 fT stores f64 = 64*(e-1) (centered softmax, scaled into
  fp8e4m3's normal range; the host adds the exact sum-of-kv-columns term
  and divides by 64).
  phase 2: partial_out = f64T.T @ kv_shard in fp8 DoubleRow (m0) plus
    three-column-group fp8 (m1), kv resident in SBUF, drained to bf16.
"""

import sys

sys.path.insert(0, "/opt/trn_rl_repo")

import numpy as np
import ml_dtypes

import concourse.bass as bass
import concourse.mybir as mybir

DT = mybir.dt
AF = mybir.ActivationFunctionType
PM = mybir.MatmulPerfMode

# problem geometry (hardcoded from the reference module)
KC, KH, KW = 128, 10, 4
H, W = 100, 64
NH, NW = H // KH, W // KW          # 10, 16
PQ = NH * NW                       # 160 q patches
D = KC * KH * KW                   # 5120
HK, WK = H - KH + 1, W - KW + 1    # 91, 61
NCORES = 8
HPC = 12                           # kv h-rows per core (8*12 = 96 >= 91)
PKC = HPC * W                      # 768 flat positions per core
ZROWS = 2 * HPC                    # 24 z rows staged per core
ZLEN = ZROWS * W                   # 1536
SCALE = 1.0 / D
LN64 = float(np.log(64.0))
F8 = ml_dtypes.float8_e4m3fn

_CACHE = {}


def _build_nc():
    nc = bass.Bass()
    zz_d = nc.declare_dram_parameter("zz", [KC, 2, ZLEN], DT.float8e4, isOutput=False)
    qm1_d = nc.declare_dram_parameter("qm1", [KC, KW, KH, 32], DT.float8e4, isOutput=False)
    idc_d = nc.declare_dram_parameter("idc", [128, 129], DT.float32, isOutput=False)
    qm0_d = nc.declare_dram_parameter("qm0", [KC, KW, KH, 128], DT.float8e4, isOutput=False)
    kv_d = nc.declare_dram_parameter("kv8", [128, 6, D], DT.float8e4, isOutput=False)
    ohi_d = nc.declare_dram_parameter("ohi", [128, D], DT.float8e4, isOutput=True)
    olo_d = nc.declare_dram_parameter("olo", [96, 4, 512], DT.float8e4, isOutput=True)
    den_d = nc.declare_dram_parameter("den", [224, 1], DT.float32, isOutput=True)

    from contextlib import ExitStack

    ctx = ExitStack()
    with ctx:
        zz_sb = ctx.enter_context(nc.sbuf_tensor([KC, 2, ZLEN], DT.float8e4))
        qm1_sb = ctx.enter_context(nc.sbuf_tensor([KC, KW, KH, 32], DT.float8e4))
        qm0_sb = ctx.enter_context(nc.sbuf_tensor([KC, KW, KH, 128], DT.float8e4))
        idc = ctx.enter_context(nc.sbuf_tensor([128, 129], DT.float32))
        kv_sb = ctx.enter_context(nc.sbuf_tensor([128, 6, D], DT.float8e4))
        e_hi = ctx.enter_context(nc.sbuf_tensor([128, PKC], DT.float32))
        e_lo = ctx.enter_context(nc.sbuf_tensor([96, 256], DT.float32))
        fT = ctx.enter_context(nc.sbuf_tensor([128, 6, PQ], DT.float8e4))
        o_hi = ctx.enter_context(nc.sbuf_tensor([128, D], DT.float8e4))
        o_lo = ctx.enter_context(nc.sbuf_tensor([96, 4, 512], DT.float8e4))
        dh_sb = ctx.enter_context(nc.sbuf_tensor([128, 1], DT.float32))
        dl_sb = ctx.enter_context(nc.sbuf_tensor([96, 1], DT.float32))
        scr = ctx.enter_context(nc.sbuf_tensor([128, 8], DT.float32))
        wz = ctx.enter_context(nc.sbuf_tensor([128, 512], DT.float8e4))

        ps_a = ctx.enter_context(nc.psum_tensor("ps_a", [128, 512], DT.float32))
        ps_b = ctx.enter_context(nc.psum_tensor("ps_b", [128, 512], DT.float32))
        ps_m = ctx.enter_context(nc.psum_tensor("ps_m", [128, 512], DT.float32))
        ps_w = ctx.enter_context(nc.psum_tensor("ps_w", [128, 512], DT.float32))
        ps_t = [
            ctx.enter_context(nc.psum_tensor(f"ps_t{i}", [128, 512], DT.float32))
            for i in range(4)
        ]

        s_z = ctx.enter_context(nc.semaphore("s_z"))
        s_z2 = ctx.enter_context(nc.semaphore("s_z2"))
        s_q1 = ctx.enter_context(nc.semaphore("s_q1"))
        s_q0 = ctx.enter_context(nc.semaphore("s_q0"))
        s_i = ctx.enter_context(nc.semaphore("s_i"))
        s_kv = [ctx.enter_context(nc.semaphore(f"s_kv{i}")) for i in range(3)]
        s_p = ctx.enter_context(nc.semaphore("s_p"))
        s_a = ctx.enter_context(nc.semaphore("s_a"))
        s_v = ctx.enter_context(nc.semaphore("s_v"))
        s_o = ctx.enter_context(nc.semaphore("s_o"))
        s_g = ctx.enter_context(nc.semaphore("s_g"))

        # p1 m1 col-groups: gA->ps_m[0:32], gB->ps_t0[32:64], gC->ps_t1[64:96]
        M1B = [ps_m, ps_t[0], ps_t[1]]
        # TR1: two 96-row transposes of e_lo halves; each yields the three
        # fT m1 chunks of its column half (col groups stack in the output)
        TR1_BANK = [ps_t[2], ps_t[3]]
        # TR0 emission order: e_hi chunks [4,5,0,1,2,3] on banks [t0,t1,t2,t3,t0,t1]
        TR0_CHUNK = [4, 5, 0, 1, 2, 3]
        TR0_BANK = [ps_t[0], ps_t[1], ps_t[2], ps_t[3], ps_a, ps_t[0]]
        TR0_SA = [4, 4, 5, 5, 5, 5]       # chunks 4,5 <- exp B; 0..3 <- exp A
        TR0_SV = [0, 0, 4, 7, 0, 8]
        # phase-2 m1 triples (n-tiles 3r..3r+2) on banks (w, b, m)
        RB = [ps_w, ps_b, ps_m]
        TRIP_SA = [4, 7, 9, 11]
        TRIP_SV = [7, 10, 16, 0]
        TRIP_KV = [[0], [1], [1, 2], [2]]

        with nc.Block() as block:

            @block.sync
            def _(sync):
                sync.dma_start(zz_sb[:, 0, :], zz_d[:, 0, :]).then_inc(s_z, 16)
                sync.dma_start(qm1_sb[:, :, :, :], qm1_d[:]).then_inc(s_q1, 16)
                sync.dma_start(zz_sb[:, 1, :], zz_d[:, 1, :]).then_inc(s_z2, 16)
                sync.dma_start(qm0_sb[:, :, :, :], qm0_d[:]).then_inc(s_q0, 16)
                sync.dma_start(idc[:, :], idc_d[:]).then_inc(s_i, 16)
                sync.dma_start(kv_sb[:, :, 0:1536], kv_d[:, :, 0:1536]).then_inc(
                    s_kv[0], 16
                )
                sync.dma_start(kv_sb[:, :, 1536:3584], kv_d[:, :, 1536:3584]).then_inc(
                    s_kv[1], 16
                )
                sync.dma_start(kv_sb[:, :, 3584:5120], kv_d[:, :, 3584:5120]).then_inc(
                    s_kv[2], 16
                )
                sync.wait_ge(s_v, 1)
                sync.dma_start(den_d[128:224, :], dl_sb[:]).then_inc(s_o, 16)
                sync.wait_ge(s_a, 7)
                sync.wait_ge(s_v, 10)
                sync.dma_start(olo_d[:, 0, :], o_lo[:, 0, :]).then_inc(s_o, 16)
                sync.wait_ge(s_v, 15)
                sync.dma_start(den_d[0:128, :], dh_sb[:]).then_inc(s_o, 16)
                sync.wait_ge(s_a, 9)
                sync.wait_ge(s_v, 16)
                sync.dma_start(olo_d[:, 1, :], o_lo[:, 1, :]).then_inc(s_o, 16)
                sync.wait_ge(s_a, 10)
                sync.wait_ge(s_v, 17)
                sync.dma_start(ohi_d[:, 0:1024], o_hi[:, 0:1024]).then_inc(s_o, 16)
                sync.wait_ge(s_a, 12)
                sync.wait_ge(s_v, 18)
                sync.dma_start(olo_d[:, 2, :], o_lo[:, 2, :]).then_inc(s_o, 16)
                sync.wait_ge(s_a, 13)
                sync.wait_ge(s_v, 19)
                sync.dma_start(ohi_d[:, 1024:2048], o_hi[:, 1024:2048]).then_inc(
                    s_o, 16
                )
                sync.wait_ge(s_a, 14)
                sync.wait_ge(s_v, 20)
                sync.dma_start(ohi_d[:, 2048:3072], o_hi[:, 2048:3072]).then_inc(
                    s_o, 16
                )
                sync.wait_ge(s_a, 15)
                sync.wait_ge(s_v, 21)
                sync.dma_start(ohi_d[:, 3072:4096], o_hi[:, 3072:4096]).then_inc(
                    s_o, 16
                )
                sync.wait_ge(s_o, 176)

            @block.tensor
            def _(pe):
                # HAM warmup on the DVE-memset tile until the z/q DMAs land
                pe.wait_ge(s_g, 1)
                for w in range(6):
                    nc.tensor.matmul(
                        ps_w[0:128, 0:512],
                        wz[:, 0:128],
                        wz[:, 0:512],
                        start=(w == 0),
                        stop=(w == 5),
                    )
                pe.wait_ge(s_z, 16)
                pe.wait_ge(s_q1, 16)
                # phase 1 m1 (q rows 128..159): 3 concurrent 32-col groups
                mfin = [None, None, None]
                for i in range(KH):
                    for j in range(KW):
                        st = i == 0 and j == 0
                        sp = i == KH - 1 and j == KW - 1
                        off = i * W + j
                        for g in range(3):
                            mfin[g] = nc.tensor.matmul(
                                M1B[g][32 * g : 32 * g + 32, 0:256],
                                qm1_sb[:, j, i, :],
                                zz_sb[:, 0, off + 256 * g : off + 256 * g + 256],
                                start=st,
                                stop=sp,
                            )
                for g in range(3):
                    mfin[g].then_inc(s_p, 1)  # s_p = 1, 2, 3
                pe.wait_ge(s_q0, 16)
                pe.wait_ge(s_z2, 16)
                # phase 1 m0 chain B (pos 512:768), DoubleRow (i, i+1) pairs
                for j in range(KW):
                    for ip in range(5):
                        st = j == 0 and ip == 0
                        sp = j == KW - 1 and ip == 4
                        off = (2 * ip) * W + j
                        mm = nc.tensor.matmul(
                            ps_b[0:128, 0:256],
                            qm0_sb[:, j, 2 * ip : 2 * ip + 2, :],
                            zz_sb[:, :, off + 512 : off + 768],
                            start=st,
                            stop=sp,
                            perf_mode=PM.DoubleRow,
                        )
                mm.then_inc(s_p, 1)  # s_p = 4
                # TR1: transpose e_lo (m1) halves -> banks t2/t3
                pe.wait_ge(s_i, 16)
                pe.wait_ge(s_a, 3)
                for k in range(2):
                    nc.tensor.matmul(
                        TR1_BANK[k][0:128, 0:96],
                        e_lo[0:96, 128 * k : 128 * k + 128],
                        idc[0:96, 0:96],
                        is_transpose=True,
                        start=True,
                        stop=True,
                    ).then_inc(s_p, 1)  # s_p = 5, 6
                # phase 1 m0 chain A (pos 0:512)
                for j in range(KW):
                    for ip in range(5):
                        st = j == 0 and ip == 0
                        sp = j == KW - 1 and ip == 4
                        off = (2 * ip) * W + j
                        mm = nc.tensor.matmul(
                            ps_a[0:128, 0:512],
                            qm0_sb[:, j, 2 * ip : 2 * ip + 2, :],
                            zz_sb[:, :, off : off + 512],
                            start=st,
                            stop=sp,
                            perf_mode=PM.DoubleRow,
                        )
                mm.then_inc(s_p, 1)  # s_p = 7

                # per-group bank-drain gates for the staged triples r1/r2:
                # gA <- ACT r(-1)-gA, gB <- ACT r(-1)-gB, gC <- DVE gc(r-1)
                STG_A = {1: 6, 2: 8}
                STG_B = {1: 7, 2: 9}
                STG_C = {1: 10, 2: 16}

                def p2triple(r):
                    staged = r in (1, 2)
                    if not staged:
                        pe.wait_ge(s_a, TRIP_SA[r])
                        if TRIP_SV[r]:
                            pe.wait_ge(s_v, TRIP_SV[r])
                    for pc in TRIP_KV[r]:
                        pe.wait_ge(s_kv[pc], 16)
                    ng = 3 if r < 3 else 1
                    mf = [None] * ng
                    for t6 in range(6):
                        st, sp = t6 == 0, t6 == 5
                        for g in range(ng):
                            if staged and t6 == 0:
                                if g == 0:
                                    pe.wait_ge(s_a, STG_A[r])
                                elif g == 1:
                                    pe.wait_ge(s_a, STG_B[r])
                                else:
                                    pe.wait_ge(s_v, STG_C[r])
                            mf[g] = nc.tensor.matmul(
                                RB[g][32 * g : 32 * g + 32, 0:512],
                                fT[:, t6, 128:160],
                                kv_sb[:, t6, (3 * r + g) * 512 : (3 * r + g + 1) * 512],
                                start=st,
                                stop=sp,
                            )
                    for g in range(ng):
                        mf[g].then_inc(s_p, 1)

                # r0 runs in the exp-A latency shadow right after chain A
                p2triple(0)  # s_p = 12, 13, 14
                # TR0: transpose e_hi chunks, exp-B-dependent chunks first
                for k in range(6):
                    c = TR0_CHUNK[k]
                    pe.wait_ge(s_a, TR0_SA[k])
                    if TR0_SV[k]:
                        pe.wait_ge(s_v, TR0_SV[k])
                    nc.tensor.matmul(
                        TR0_BANK[k][0:128, 0:128],
                        e_hi[:, c * 128 : (c + 1) * 128],
                        idc[0:128, 0:128],
                        is_transpose=True,
                        start=True,
                        stop=True,
                    ).then_inc(s_p, 1)  # s_p = 11..16

                def p2pair(k, banks, sa, sv, kvs, sv2=0):
                    if sa:
                        pe.wait_ge(s_a, sa)
                    pe.wait_ge(s_v, sv)
                    for pc in kvs:
                        pe.wait_ge(s_kv[pc], 16)
                    bA, bB = ps_t[banks[0]], ps_t[banks[1]]
                    for tp in range(3):
                        if tp == 1 and sv2:
                            pe.wait_ge(s_v, sv2)
                        st, sp = tp == 0, tp == 2
                        mA = nc.tensor.matmul(
                            bA[0:128, 0:512],
                            fT[:, 2 * tp : 2 * tp + 2, 0:128],
                            kv_sb[
                                :, 2 * tp : 2 * tp + 2, (2 * k) * 512 : (2 * k + 1) * 512
                            ],
                            start=st,
                            stop=sp,
                            perf_mode=PM.DoubleRow,
                        )
                        mB = nc.tensor.matmul(
                            bB[0:128, 0:512],
                            fT[:, 2 * tp : 2 * tp + 2, 0:128],
                            kv_sb[
                                :,
                                2 * tp : 2 * tp + 2,
                                (2 * k + 1) * 512 : (2 * k + 2) * 512,
                            ],
                            start=st,
                            stop=sp,
                            perf_mode=PM.DoubleRow,
                        )
                    mA.then_inc(s_p, 1)
                    mB.then_inc(s_p, 1)

                p2triple(1)                       # s_p = 21, 22, 23
                p2pair(0, (2, 3), 0, 12, [0], sv2=14)  # s_p = 20, 21
                p2triple(2)                       # s_p = 26, 27, 28
                p2pair(1, (0, 1), 0, 14, [0, 1])  # s_p = 25, 26
                p2pair(2, (2, 3), 10, 17, [1])    # s_p = 27, 28
                p2pair(3, (0, 1), 13, 19, [1, 2])  # s_p = 29, 30
                p2triple(3)                       # s_p = 35
                p2pair(4, (2, 3), 14, 20, [2])    # s_p = 32, 33

            @block.scalar
            def _(act):
                # warm the exp table set early (reads the memset tile)
                act.wait_ge(s_g, 1)
                nc.scalar.activation(
                    scr[:, :], wz[:, 0:8], AF.Exp, bias=0.0, scale=1.0
                )
                act.wait_ge(s_i, 16)
                # e64 = 64 * exp(s * SCALE); bias AP holds ln(64)
                for g in range(3):
                    act.wait_ge(s_p, 1 + g)
                    nc.scalar.activation(
                        e_lo[32 * g : 32 * g + 32, 0:256],
                        M1B[g][32 * g : 32 * g + 32, 0:256],
                        AF.Exp,
                        bias=idc[32 * g : 32 * g + 32, 128:129],
                        scale=SCALE,
                    ).then_inc(s_a, 1)  # 1, 2, 3
                act.wait_ge(s_p, 4)
                nc.scalar.activation(
                    e_hi[:, 512:768], ps_b[0:128, 0:256], AF.Exp,
                    bias=idc[:, 128:129], scale=SCALE,
                ).then_inc(s_a, 1)  # 4 (exp B)
                act.wait_ge(s_p, 7)
                nc.scalar.activation(
                    e_hi[:, 0:512], ps_a[0:128, 0:512], AF.Exp,
                    bias=idc[:, 128:129], scale=SCALE,
                ).then_inc(s_a, 1)  # 5 (exp A)

                def m1drain(gi, r, spv):
                    act.wait_ge(s_p, spv)
                    nc.scalar.activation(
                        o_lo[32 * gi : 32 * gi + 32, r, :],
                        RB[gi][32 * gi : 32 * gi + 32, 0:512],
                        AF.Copy,
                    ).then_inc(s_a, 1)

                def ntdrain_a(g, spv, bank):
                    act.wait_ge(s_p, spv)
                    nc.scalar.activation(
                        o_hi[:, g * 512 : (g + 1) * 512],
                        bank[0:128, 0:512],
                        AF.Copy,
                    ).then_inc(s_a, 1)

                m1drain(0, 0, 8)             # 6
                m1drain(1, 0, 9)             # 7
                m1drain(0, 1, 17)            # 8
                m1drain(1, 1, 18)            # 9
                ntdrain_a(1, 21, ps_t[3])    # 10
                m1drain(0, 2, 22)            # 11
                m1drain(1, 2, 23)            # 12
                ntdrain_a(3, 26, ps_t[1])    # 13
                ntdrain_a(5, 28, ps_t[3])    # 14
                ntdrain_a(7, 30, ps_t[1])    # 15
                m1drain(0, 3, 31)            # 16
                act.wait_ge(s_a, 16)
                nc.scalar.dma_start(olo_d[0:32, 3, :], o_lo[0:32, 3, :]).then_inc(
                    s_o, 16
                )
                ntdrain_a(9, 33, ps_t[3])    # 17
                act.wait_ge(s_a, 17)
                act.wait_ge(s_v, 22)
                nc.scalar.dma_start(
                    ohi_d[:, 4096:5120], o_hi[:, 4096:5120]
                ).then_inc(s_o, 16)

            @block.vector
            def _(dve):
                nc.vector.memset(wz[:, :], 0.0).then_inc(s_g, 1)
                dve.wait_ge(s_a, 3)
                nc.vector.reduce_sum(
                    dl_sb[:], e_lo[:, :], axis=mybir.AxisListType.X
                ).then_inc(s_v, 1)  # 1
                # fT m1 copies: f64 = e64T - 64, cast to fp8
                for k in range(2):
                    dve.wait_ge(s_p, 5 + k)
                    for g in range(3):
                        nc.vector.tensor_scalar_sub(
                            fT[:, 2 * g + k, 128:160],
                            TR1_BANK[k][0:128, 32 * g : 32 * g + 32],
                            64.0,
                        ).then_inc(s_v, 1)  # 2..7

                def gc_drain(r, sp_val):
                    dve.wait_ge(s_p, sp_val)
                    nc.vector.tensor_copy(
                        o_lo[64:96, r, :], ps_m[64:96, 0:512]
                    ).then_inc(s_v, 1)

                def nt_drain(g, sp_val, bank):
                    dve.wait_ge(s_p, sp_val)
                    nc.vector.tensor_copy(
                        o_hi[:, g * 512 : (g + 1) * 512], bank[0:128, 0:512]
                    ).then_inc(s_v, 1)

                dve.wait_ge(s_p, 12)
                for k in range(2):
                    nc.vector.tensor_scalar_sub(
                        fT[:, TR0_CHUNK[k], 0:128], TR0_BANK[k][0:128, 0:128], 64.0
                    ).then_inc(s_v, 1)  # 8, 9
                gc_drain(0, 10)            # 10
                dve.wait_ge(s_p, 14)
                for k in range(2, 4):
                    nc.vector.tensor_scalar_sub(
                        fT[:, TR0_CHUNK[k], 0:128], TR0_BANK[k][0:128, 0:128], 64.0
                    ).then_inc(s_v, 1)  # 11, 12
                dve.wait_ge(s_p, 16)
                for k in range(4, 6):
                    nc.vector.tensor_scalar_sub(
                        fT[:, TR0_CHUNK[k], 0:128], TR0_BANK[k][0:128, 0:128], 64.0
                    ).then_inc(s_v, 1)  # 13, 14
                dve.wait_ge(s_a, 5)
                nc.vector.reduce_sum(
                    dh_sb[:], e_hi[:, :], axis=mybir.AxisListType.X
                ).then_inc(s_v, 1)  # 15
                gc_drain(1, 19)            # 16
                nt_drain(0, 20, ps_t[2])   # 17
                gc_drain(2, 24)            # 18
                nt_drain(2, 25, ps_t[0])   # 19
                nt_drain(4, 27, ps_t[2])   # 20
                nt_drain(6, 29, ps_t[0])   # 21
                nt_drain(8, 32, ps_t[2])   # 22

    return nc


def _host_prep(z1_hat, z2):
    z1 = np.asarray(z1_hat, dtype=np.float32)[0]  # [128, 100, 64]
    z2a = np.asarray(z2, dtype=np.float32)[0]

    # q patches [160, 5120]; device layout q8 [128, j, i, 160]
    q = z1.reshape(KC, NH, KH, NW, KW).transpose(1, 3, 0, 2, 4).reshape(PQ, D)
    q4 = q.reshape(PQ, KC, KH, KW)
    q8 = q4.transpose(1, 3, 2, 0).astype(F8)  # [c, j, i, p]
    qm0 = np.ascontiguousarray(q8[:, :, :, 0:128])
    qm1 = np.ascontiguousarray(q8[:, :, :, 128:160])

    # padded z2: rows 100..111 zero
    z_pad = np.zeros((KC, 112, W), dtype=np.float32)
    z_pad[:, :H] = z2a
    z8_pad = z_pad.astype(F8)

    # sliding kv patches from padded z2 (original fp32 values, cast per-row)
    sw = np.lib.stride_tricks.sliding_window_view(z_pad, (KH, KW), axis=(1, 2))

    q64 = q.astype(np.float64)
    ij_off = (np.arange(KH)[:, None] * W + np.arange(KW)[None, :]).reshape(-1)  # [40]

    idc = np.zeros((128, 129), dtype=np.float32)
    idc[:, 0:128] = np.eye(128, dtype=np.float32)
    idc[:, 128] = LN64

    in_maps = []
    corrs = []
    for core in range(NCORES):
        h0 = HPC * core
        slab8 = z8_pad[:, h0 : h0 + ZROWS, :].reshape(KC, ZLEN)
        zz = np.zeros((KC, 2, ZLEN), dtype=F8)
        zz[:, 0, :] = slab8
        zz[:, 1, 0 : ZLEN - W] = slab8[:, W:]
        # kv rows indexed by flat position p = h_local*64 + w
        kvp = np.zeros((PKC, D), dtype=F8)
        hh = np.arange(PKC) // W
        ww = np.arange(PKC) % W
        real = (ww < WK) & (h0 + hh < HK)
        ridx = np.nonzero(real)[0]
        kvp[ridx] = (
            sw[:, h0 + hh[ridx], ww[ridx]].transpose(1, 0, 2, 3).reshape(-1, D)
        ).astype(F8)
        kv8 = np.ascontiguousarray(kvp.reshape(6, 128, D).transpose(1, 0, 2))
        in_maps.append(
            {"zz": zz, "qm0": qm0, "qm1": qm1, "kv8": kv8, "idc": idc}
        )
        # denominator correction: computed-but-invalid stream positions,
        # recomputed exactly (fp64) from the original values.
        bad = np.nonzero(~real)[0]
        zf = z_pad[:, h0 : h0 + ZROWS, :].reshape(KC, ZLEN).astype(np.float64)
        win = zf[:, bad[:, None] + ij_off[None, :]]  # [128, nb, 40]
        patches = win.transpose(1, 0, 2).reshape(len(bad), D)
        s_bad = q64 @ patches.T  # [160, nb]
        corrs.append(np.exp(s_bad * SCALE).sum(axis=1))

    corr = np.sum(corrs, axis=0)
    swr = sw[:, :HK, :WK]
    colsum = swr.astype(np.float64).sum(axis=(1, 2)).reshape(D)  # [5120]
    return in_maps, corr, colsum


def kernel(z1_hat, z2):
    from concourse.bass_utils import run_bass_kernel_spmd

    in_maps, corr, colsum = _host_prep(z1_hat, z2)
    if "nc" not in _CACHE:
        _CACHE["nc"] = _build_nc()
    nc = _CACHE["nc"]
    res = run_bass_kernel_spmd(nc, in_maps, list(range(NCORES)))
    num = np.broadcast_to(colsum, (PQ, D)).astype(np.float64).copy()
    den = -corr
    for r in res.results:
        ohi = r["ohi"].astype(np.float64)  # [128, 5120] = 64 * partial m0
        olo = r["olo"].astype(np.float64).reshape(96, 4, 512)
        part = np.zeros((PQ, D))
        part[0:128] = ohi
        for nt in range(10):
            rr, g = nt // 3, nt % 3
            part[128:160, nt * 512 : (nt + 1) * 512] = olo[32 * g : 32 * g + 32, rr]
        num += part / 64.0
        dv = r["den"].astype(np.float64)[:, 0] / 64.0
        den = den + np.concatenate(
            [dv[0:128], dv[128:160] + dv[160:192] + dv[192:224]]
        )
    out = (num / den[:, None]).astype(np.float32)
    # fold patches back: [160, 5120] -> [1, 128, 100, 64]
    out = out.reshape(NH, NW, KC, KH, KW).transpose(2, 0, 3, 1, 4)
    return np.ascontiguousarray(out.reshape(1, KC, H, W))


# revision 38
# speedup vs baseline: 1.0294x; 1.0152x over previous
"""Trainium2 Bass kernel for BottleneckAttention (patch attention), fp8 edition.

q patches [160, 5120] from z1_hat (non-overlapping 10x4 unfold),
kv patches [5551, 5120] from z2 (overlapping unfold, Hk=91 x Wk=61),
scores = q @ kv.T / 5120, softmax over kv patches, out = attn @ kv,
folded back to [1, 128, 100, 64].

Sharding: contiguous blocks of 12 kv h-rows per core (8 x 12 = 96 >= 91).
Each core owns the 768 flat positions p = h_local*64 + w (w in [0,64));
positions with w >= 61 or h >= 91 are invalid -- their kv rows are zeroed
so they never touch the numerator, and the host subtracts their exactly
recomputed exp contribution from the denominator. Every core computes all
160 q rows; the host combines with an all-gather softmax.

Per-core kernel (raw Bass, explicit semaphores), fp8e4m3 everywhere on
the PE with DoubleRow (K=256) perf mode for the M=128 matmul blocks:
  phase 1: scores as implicit convolution against the SBUF-resident
    z2 slab (zz holds the slab plus a 64-shifted copy so (i, i+1) kernel
    row pairs form clean [128, 2, N] DoubleRow moving operands).
    q rows 0..127 run M=128 DoubleRow; rows 128..159 run as three
    concurrent 32-wide PE column groups (no DoubleRow -- col tiling and
    DoubleRow are mutually exclusive).
  exp on ScalarE with scale=1/5120 and bias=ln(64): e64 = 64*exp(s).
  row-sum denominator (64x) on VectorE; host divides by 64.
  PE transposes of e64 chunks; the ACT psum->sbuf copy applies bias=-64
  so the fp8 fT stores f64 = 64*(e-1) (centered softmax, scaled into
  fp8e4m3's normal range; the host adds the exact sum-of-kv-columns term
  and divides by 64).
  phase 2: partial_out = f64T.T @ kv_shard in fp8 DoubleRow (m0) plus
    three-column-group fp8 (m1), kv resident in SBUF, drained to bf16.
"""

import sys

sys.path.insert(0, "/opt/trn_rl_repo")

import numpy as np
import ml_dtypes

import concourse.bass as bass
import concourse.mybir as mybir

DT = mybir.dt
AF = mybir.ActivationFunctionType
PM = mybir.MatmulPerfMode

# problem geometry (hardcoded from the reference module)
KC, KH, KW = 128, 10, 4
H, W = 100, 64
NH, NW = H // KH, W // KW          # 10, 16
PQ = NH * NW                       # 160 q patches
D = KC * KH * KW                   # 5120
HK, WK = H - KH + 1, W - KW + 1    # 91, 61
NCORES = 8
HPC = 12                           # kv h-rows per core (8*12 = 96 >= 91)
PKC = HPC * W                      # 768 flat positions per core
ZROWS = 2 * HPC                    # 24 z rows staged per core
ZLEN = ZROWS * W                   # 1536
SCALE = 1.0 / D
LN64 = float(np.log(64.0))
F8 = ml_dtypes.float8_e4m3fn

_CACHE = {}


def _build_nc():
    nc = bass.Bass()
    zz_d = nc.declare_dram_parameter("zz", [KC, 2, ZLEN], DT.float8e4, isOutput=False)
    qm1_d = nc.declare_dram_parameter("qm1", [KC, KW, KH, 32], DT.float8e4, isOutput=False)
    idc_d = nc.declare_dram_parameter("idc", [128, 129], DT.float32, isOutput=False)
    qm0_d = nc.declare_dram_parameter("qm0", [KC, KW, KH, 128], DT.float8e4, isOutput=False)
    kv_d = nc.declare_dram_parameter("kv8", [128, 6, D], DT.float8e4, isOutput=False)
    ohi_d = nc.declare_dram_parameter("ohi", [128, D], DT.float8e4, isOutput=True)
    olo_d = nc.declare_dram_parameter("olo", [96, 4, 512], DT.float8e4, isOutput=True)
    den_d = nc.declare_dram_parameter("den", [224, 1], DT.float32, isOutput=True)

    from contextlib import ExitStack

    ctx = ExitStack()
    with ctx:
        zz_sb = ctx.enter_context(nc.sbuf_tensor([KC, 2, ZLEN], DT.float8e4))
        qm1_sb = ctx.enter_context(nc.sbuf_tensor([KC, KW, KH, 32], DT.float8e4))
        qm0_sb = ctx.enter_context(nc.sbuf_tensor([KC, KW, KH, 128], DT.float8e4))
        idc = ctx.enter_context(nc.sbuf_tensor([128, 129], DT.float32))
        kv_sb = ctx.enter_context(nc.sbuf_tensor([128, 6, D], DT.float8e4))
        e_hi = ctx.enter_context(nc.sbuf_tensor([128, PKC], DT.float32))
        e_lo = ctx.enter_context(nc.sbuf_tensor([96, 256], DT.float32))
        fT = ctx.enter_context(nc.sbuf_tensor([128, 6, PQ], DT.float8e4))
        o_hi = ctx.enter_context(nc.sbuf_tensor([128, D], DT.float8e4))
        o_lo = ctx.enter_context(nc.sbuf_tensor([96, 4, 512], DT.float8e4))
        dh_sb = ctx.enter_context(nc.sbuf_tensor([128, 1], DT.float32))
        dl_sb = ctx.enter_context(nc.sbuf_tensor([96, 1], DT.float32))
        scr = ctx.enter_context(nc.sbuf_tensor([128, 8], DT.float32))
        wz = ctx.enter_context(nc.sbuf_tensor([128, 512], DT.float8e4))

        ps_a = ctx.enter_context(nc.psum_tensor("ps_a", [128, 512], DT.float32))
        ps_b = ctx.enter_context(nc.psum_tensor("ps_b", [128, 512], DT.float32))
        ps_m = ctx.enter_context(nc.psum_tensor("ps_m", [128, 512], DT.float32))
        ps_w = ctx.enter_context(nc.psum_tensor("ps_w", [128, 512], DT.float32))
        ps_t = [
            ctx.enter_context(nc.psum_tensor(f"ps_t{i}", [128, 512], DT.float32))
            for i in range(4)
        ]

        s_z = ctx.enter_context(nc.semaphore("s_z"))
        s_z2 = ctx.enter_context(nc.semaphore("s_z2"))
        s_q1 = ctx.enter_context(nc.semaphore("s_q1"))
        s_q0 = ctx.enter_context(nc.semaphore("s_q0"))
        s_i = ctx.enter_context(nc.semaphore("s_i"))
        s_kv = [ctx.enter_context(nc.semaphore(f"s_kv{i}")) for i in range(3)]
        s_p = ctx.enter_context(nc.semaphore("s_p"))
        s_a = ctx.enter_context(nc.semaphore("s_a"))
        s_v = ctx.enter_context(nc.semaphore("s_v"))
        s_o = ctx.enter_context(nc.semaphore("s_o"))
        s_g = ctx.enter_context(nc.semaphore("s_g"))

        # p1 m1 col-groups: gA->ps_m[0:32], gB->ps_t0[32:64], gC->ps_t1[64:96]
        M1B = [ps_m, ps_t[0], ps_t[1]]
        # TR1: two 96-row transposes of e_lo halves; each yields the three
        # fT m1 chunks of its column half (col groups stack in the output)
        TR1_BANK = [ps_t[2], ps_t[3]]
        # TR0 emission order: e_hi chunks [4,5,0,1,2,3] on banks [t0,t1,t2,t3,t0,t1]
        TR0_CHUNK = [4, 5, 0, 1, 2, 3]
        TR0_BANK = [ps_t[0], ps_t[1], ps_t[2], ps_t[3], ps_a, ps_t[0]]
        TR0_SA = [4, 4, 5, 5, 5, 5]       # chunks 4,5 <- exp B; 0..3 <- exp A
        TR0_SV = [0, 0, 4, 7, 0, 8]
        # phase-2 m1 triples (n-tiles 3r..3r+2) on banks (w, b, m)
        RB = [ps_w, ps_b, ps_m]
        TRIP_SA = [4, 7, 9, 11]
        TRIP_SV = [7, 10, 16, 0]
        TRIP_KV = [[0], [1], [1, 2], [2]]

        with nc.Block() as block:

            @block.sync
            def _(sync):
                sync.dma_start(zz_sb[:, 0, :], zz_d[:, 0, :]).then_inc(s_z, 16)
                sync.dma_start(qm1_sb[:, :, :, :], qm1_d[:]).then_inc(s_q1, 16)
                sync.dma_start(zz_sb[:, 1, :], zz_d[:, 1, :]).then_inc(s_z2, 16)
                sync.dma_start(qm0_sb[:, :, :, :], qm0_d[:]).then_inc(s_q0, 16)
                sync.dma_start(idc[:, :], idc_d[:]).then_inc(s_i, 16)
                sync.dma_start(kv_sb[:, :, 0:1536], kv_d[:, :, 0:1536]).then_inc(
                    s_kv[0], 16
                )
                sync.dma_start(kv_sb[:, :, 1536:3584], kv_d[:, :, 1536:3584]).then_inc(
                    s_kv[1], 16
                )
                sync.dma_start(kv_sb[:, :, 3584:5120], kv_d[:, :, 3584:5120]).then_inc(
                    s_kv[2], 16
                )
                sync.wait_ge(s_v, 1)
                sync.dma_start(den_d[128:224, :], dl_sb[:]).then_inc(s_o, 16)
                sync.wait_ge(s_a, 7)
                sync.wait_ge(s_v, 10)
                sync.dma_start(olo_d[:, 0, :], o_lo[:, 0, :]).then_inc(s_o, 16)
                sync.wait_ge(s_v, 15)
                sync.dma_start(den_d[0:128, :], dh_sb[:]).then_inc(s_o, 16)
                sync.wait_ge(s_a, 9)
                sync.wait_ge(s_v, 16)
                sync.dma_start(olo_d[:, 1, :], o_lo[:, 1, :]).then_inc(s_o, 16)
                sync.wait_ge(s_a, 10)
                sync.wait_ge(s_v, 17)
                sync.dma_start(ohi_d[:, 0:1024], o_hi[:, 0:1024]).then_inc(s_o, 16)
                sync.wait_ge(s_a, 12)
                sync.wait_ge(s_v, 18)
                sync.dma_start(olo_d[:, 2, :], o_lo[:, 2, :]).then_inc(s_o, 16)
                sync.wait_ge(s_a, 13)
                sync.wait_ge(s_v, 19)
                sync.dma_start(ohi_d[:, 1024:2048], o_hi[:, 1024:2048]).then_inc(
                    s_o, 16
                )
                sync.wait_ge(s_a, 14)
                sync.wait_ge(s_v, 20)
                sync.dma_start(ohi_d[:, 2048:3072], o_hi[:, 2048:3072]).then_inc(
                    s_o, 16
                )
                sync.wait_ge(s_a, 15)
                sync.wait_ge(s_v, 21)
                sync.dma_start(ohi_d[:, 3072:4096], o_hi[:, 3072:4096]).then_inc(
                    s_o, 16
                )
                sync.wait_ge(s_o, 176)

            @block.tensor
            def _(pe):
                # HAM warmup on the DVE-memset tile until the z/q DMAs land
                pe.wait_ge(s_g, 1)
                for w in range(6):
                    nc.tensor.matmul(
                        ps_w[0:128, 0:512],
                        wz[:, 0:128],
                        wz[:, 0:512],
                        start=(w == 0),
                        stop=(w == 5),
                    )
                pe.wait_ge(s_z, 16)
                pe.wait_ge(s_q1, 16)
                # phase 1 m1 (q rows 128..159): 3 concurrent 32-col groups
                mfin = [None, None, None]
                for i in range(KH):
                    for j in range(KW):
                        st = i == 0 and j == 0
                        sp = i == KH - 1 and j == KW - 1
                        off = i * W + j
                        for g in range(3):
                            mfin[g] = nc.tensor.matmul(
                                M1B[g][32 * g : 32 * g + 32, 0:256],
                                qm1_sb[:, j, i, :],
                                zz_sb[:, 0, off + 256 * g : off + 256 * g + 256],
                                start=st,
                                stop=sp,
                            )
                for g in range(3):
                    mfin[g].then_inc(s_p, 1)  # s_p = 1, 2, 3
                pe.wait_ge(s_q0, 16)
                pe.wait_ge(s_z2, 16)
                # phase 1 m0 chain B (pos 512:768), DoubleRow (i, i+1) pairs
                for j in range(KW):
                    for ip in range(5):
                        st = j == 0 and ip == 0
                        sp = j == KW - 1 and ip == 4
                        off = (2 * ip) * W + j
                        mm = nc.tensor.matmul(
                            ps_b[0:128, 0:256],
                            qm0_sb[:, j, 2 * ip : 2 * ip + 2, :],
                            zz_sb[:, :, off + 512 : off + 768],
                            start=st,
                            stop=sp,
                            perf_mode=PM.DoubleRow,
                        )
                mm.then_inc(s_p, 1)  # s_p = 4
                # TR1: transpose e_lo (m1) halves -> banks t2/t3
                pe.wait_ge(s_i, 16)
                pe.wait_ge(s_a, 3)
                for k in range(2):
                    nc.tensor.matmul(
                        TR1_BANK[k][0:128, 0:96],
                        e_lo[0:96, 128 * k : 128 * k + 128],
                        idc[0:96, 0:96],
                        is_transpose=True,
                        start=True,
                        stop=True,
                    ).then_inc(s_p, 1)  # s_p = 5, 6
                # phase 1 m0 chain A (pos 0:512)
                for j in range(KW):
                    for ip in range(5):
                        st = j == 0 and ip == 0
                        sp = j == KW - 1 and ip == 4
                        off = (2 * ip) * W + j
                        mm = nc.tensor.matmul(
                            ps_a[0:128, 0:512],
                            qm0_sb[:, j, 2 * ip : 2 * ip + 2, :],
                            zz_sb[:, :, off : off + 512],
                            start=st,
                            stop=sp,
                            perf_mode=PM.DoubleRow,
                        )
                mm.then_inc(s_p, 1)  # s_p = 7

                # per-group bank-drain gates for the staged triples r1/r2:
                # gA <- ACT r(-1)-gA, gB <- ACT r(-1)-gB, gC <- DVE gc(r-1)
                STG_A = {1: 6, 2: 8}
                STG_B = {1: 7, 2: 9}
                STG_C = {1: 10, 2: 16}

                def p2triple(r):
                    staged = r in (1, 2)
                    if not staged:
                        pe.wait_ge(s_a, TRIP_SA[r])
                        if TRIP_SV[r]:
                            pe.wait_ge(s_v, TRIP_SV[r])
                    for pc in TRIP_KV[r]:
                        pe.wait_ge(s_kv[pc], 16)
                    ng = 3 if r < 3 else 1
                    mf = [None] * ng
                    for t6 in range(6):
                        st, sp = t6 == 0, t6 == 5
                        for g in range(ng):
                            if staged and t6 == 0:
                                if g == 0:
                                    pe.wait_ge(s_a, STG_A[r])
                                elif g == 1:
                                    pe.wait_ge(s_a, STG_B[r])
                                else:
                                    pe.wait_ge(s_v, STG_C[r])
                            mf[g] = nc.tensor.matmul(
                                RB[g][32 * g : 32 * g + 32, 0:512],
                                fT[:, t6, 128:160],
                                kv_sb[:, t6, (3 * r + g) * 512 : (3 * r + g + 1) * 512],
                                start=st,
                                stop=sp,
                            )
                    for g in range(ng):
                        mf[g].then_inc(s_p, 1)

                # r0 runs in the exp-A latency shadow right after chain A
                p2triple(0)  # s_p = 12, 13, 14
                # TR0: transpose e_hi chunks, exp-B-dependent chunks first
                for k in range(6):
                    c = TR0_CHUNK[k]
                    pe.wait_ge(s_a, TR0_SA[k])
                    if TR0_SV[k]:
                        pe.wait_ge(s_v, TR0_SV[k])
                    nc.tensor.matmul(
                        TR0_BANK[k][0:128, 0:128],
                        e_hi[:, c * 128 : (c + 1) * 128],
                        idc[0:128, 0:128],
                        is_transpose=True,
                        start=True,
                        stop=True,
                    ).then_inc(s_p, 1)  # s_p = 11..16

                def p2pair(k, banks, sa, sv, kvs, sv2=0, staged=False):
                    if not staged:
                        if sa:
                            pe.wait_ge(s_a, sa)
                        pe.wait_ge(s_v, sv)
                    for pc in kvs:
                        pe.wait_ge(s_kv[pc], 16)
                    bA, bB = ps_t[banks[0]], ps_t[banks[1]]
                    for tp in range(3):
                        if tp == 1 and sv2:
                            pe.wait_ge(s_v, sv2)
                        if staged and tp == 0:
                            pe.wait_ge(s_v, sv)
                        st, sp = tp == 0, tp == 2
                        mA = nc.tensor.matmul(
                            bA[0:128, 0:512],
                            fT[:, 2 * tp : 2 * tp + 2, 0:128],
                            kv_sb[
                                :, 2 * tp : 2 * tp + 2, (2 * k) * 512 : (2 * k + 1) * 512
                            ],
                            start=st,
                            stop=sp,
                            perf_mode=PM.DoubleRow,
                        )
                        if staged and tp == 0:
                            pe.wait_ge(s_a, sa)
                        mB = nc.tensor.matmul(
                            bB[0:128, 0:512],
                            fT[:, 2 * tp : 2 * tp + 2, 0:128],
                            kv_sb[
                                :,
                                2 * tp : 2 * tp + 2,
                                (2 * k + 1) * 512 : (2 * k + 2) * 512,
                            ],
                            start=st,
                            stop=sp,
                            perf_mode=PM.DoubleRow,
                        )
                    mA.then_inc(s_p, 1)
                    mB.then_inc(s_p, 1)

                p2triple(1)                       # s_p = 21, 22, 23
                p2pair(0, (2, 3), 0, 12, [0], sv2=14)  # s_p = 20, 21
                p2triple(2)                       # s_p = 26, 27, 28
                p2pair(1, (0, 1), 0, 14, [0, 1])  # s_p = 25, 26
                p2pair(2, (2, 3), 10, 17, [1], staged=True)    # s_p = 27, 28
                p2pair(3, (0, 1), 13, 19, [1, 2], staged=True)  # s_p = 29, 30
                p2triple(3)                       # s_p = 35
                p2pair(4, (2, 3), 14, 20, [2], staged=True)    # s_p = 32, 33

            @block.scalar
            def _(act):
                # warm the exp table set early (reads the memset tile)
                act.wait_ge(s_g, 1)
                nc.scalar.activation(
                    scr[:, :], wz[:, 0:8], AF.Exp, bias=0.0, scale=1.0
                )
                act.wait_ge(s_i, 16)
                # e64 = 64 * exp(s * SCALE); bias AP holds ln(64)
                for g in range(3):
                    act.wait_ge(s_p, 1 + g)
                    nc.scalar.activation(
                        e_lo[32 * g : 32 * g + 32, 0:256],
                        M1B[g][32 * g : 32 * g + 32, 0:256],
                        AF.Exp,
                        bias=idc[32 * g : 32 * g + 32, 128:129],
                        scale=SCALE,
                    ).then_inc(s_a, 1)  # 1, 2, 3
                act.wait_ge(s_p, 4)
                nc.scalar.activation(
                    e_hi[:, 512:768], ps_b[0:128, 0:256], AF.Exp,
                    bias=idc[:, 128:129], scale=SCALE,
                ).then_inc(s_a, 1)  # 4 (exp B)
                act.wait_ge(s_p, 7)
                nc.scalar.activation(
                    e_hi[:, 0:512], ps_a[0:128, 0:512], AF.Exp,
                    bias=idc[:, 128:129], scale=SCALE,
                ).then_inc(s_a, 1)  # 5 (exp A)

                def m1drain(gi, r, spv):
                    act.wait_ge(s_p, spv)
                    nc.scalar.activation(
                        o_lo[32 * gi : 32 * gi + 32, r, :],
                        RB[gi][32 * gi : 32 * gi + 32, 0:512],
                        AF.Copy,
                    ).then_inc(s_a, 1)

                def ntdrain_a(g, spv, bank):
                    act.wait_ge(s_p, spv)
                    nc.scalar.activation(
                        o_hi[:, g * 512 : (g + 1) * 512],
                        bank[0:128, 0:512],
                        AF.Copy,
                    ).then_inc(s_a, 1)

                m1drain(0, 0, 8)             # 6
                m1drain(1, 0, 9)             # 7
                m1drain(0, 1, 17)            # 8
                m1drain(1, 1, 18)            # 9
                ntdrain_a(1, 21, ps_t[3])    # 10
                m1drain(0, 2, 22)            # 11
                m1drain(1, 2, 23)            # 12
                ntdrain_a(3, 26, ps_t[1])    # 13
                ntdrain_a(5, 28, ps_t[3])    # 14
                ntdrain_a(7, 30, ps_t[1])    # 15
                m1drain(0, 3, 31)            # 16
                act.wait_ge(s_a, 16)
                nc.scalar.dma_start(olo_d[0:32, 3, :], o_lo[0:32, 3, :]).then_inc(
                    s_o, 16
                )
                ntdrain_a(9, 33, ps_t[3])    # 17
                act.wait_ge(s_a, 17)
                act.wait_ge(s_v, 22)
                nc.scalar.dma_start(
                    ohi_d[:, 4096:5120], o_hi[:, 4096:5120]
                ).then_inc(s_o, 16)

            @block.vector
            def _(dve):
                nc.vector.memset(wz[:, :], 0.0).then_inc(s_g, 1)
                dve.wait_ge(s_a, 3)
                nc.vector.reduce_sum(
                    dl_sb[:], e_lo[:, :], axis=mybir.AxisListType.X
                ).then_inc(s_v, 1)  # 1
                # fT m1 copies: f64 = e64T - 64, cast to fp8
                for k in range(2):
                    dve.wait_ge(s_p, 5 + k)
                    for g in range(3):
                        nc.vector.tensor_scalar_sub(
                            fT[:, 2 * g + k, 128:160],
                            TR1_BANK[k][0:128, 32 * g : 32 * g + 32],
                            64.0,
                        ).then_inc(s_v, 1)  # 2..7

                def gc_drain(r, sp_val):
                    dve.wait_ge(s_p, sp_val)
                    nc.vector.tensor_copy(
                        o_lo[64:96, r, :], ps_m[64:96, 0:512]
                    ).then_inc(s_v, 1)

                def nt_drain(g, sp_val, bank):
                    dve.wait_ge(s_p, sp_val)
                    nc.vector.tensor_copy(
                        o_hi[:, g * 512 : (g + 1) * 512], bank[0:128, 0:512]
                    ).then_inc(s_v, 1)

                dve.wait_ge(s_p, 12)
                for k in range(2):
                    nc.vector.tensor_scalar_sub(
                        fT[:, TR0_CHUNK[k], 0:128], TR0_BANK[k][0:128, 0:128], 64.0
                    ).then_inc(s_v, 1)  # 8, 9
                gc_drain(0, 10)            # 10
                dve.wait_ge(s_p, 14)
                for k in range(2, 4):
                    nc.vector.tensor_scalar_sub(
                        fT[:, TR0_CHUNK[k], 0:128], TR0_BANK[k][0:128, 0:128], 64.0
                    ).then_inc(s_v, 1)  # 11, 12
                dve.wait_ge(s_p, 16)
                for k in range(4, 6):
                    nc.vector.tensor_scalar_sub(
                        fT[:, TR0_CHUNK[k], 0:128], TR0_BANK[k][0:128, 0:128], 64.0
                    ).then_inc(s_v, 1)  # 13, 14
                dve.wait_ge(s_a, 5)
                nc.vector.reduce_sum(
                    dh_sb[:], e_hi[:, :], axis=mybir.AxisListType.X
                ).then_inc(s_v, 1)  # 15
                gc_drain(1, 19)            # 16
                nt_drain(0, 20, ps_t[2])   # 17
                gc_drain(2, 24)            # 18
                nt_drain(2, 25, ps_t[0])   # 19
                nt_drain(4, 27, ps_t[2])   # 20
                nt_drain(6, 29, ps_t[0])   # 21
                nt_drain(8, 32, ps_t[2])   # 22

    return nc


def _host_prep(z1_hat, z2):
    z1 = np.asarray(z1_hat, dtype=np.float32)[0]  # [128, 100, 64]
    z2a = np.asarray(z2, dtype=np.float32)[0]

    # q patches [160, 5120]; device layout q8 [128, j, i, 160]
    q = z1.reshape(KC, NH, KH, NW, KW).transpose(1, 3, 0, 2, 4).reshape(PQ, D)
    q4 = q.reshape(PQ, KC, KH, KW)
    q8 = q4.transpose(1, 3, 2, 0).astype(F8)  # [c, j, i, p]
    qm0 = np.ascontiguousarray(q8[:, :, :, 0:128])
    qm1 = np.ascontiguousarray(q8[:, :, :, 128:160])

    # padded z2: rows 100..111 zero
    z_pad = np.zeros((KC, 112, W), dtype=np.float32)
    z_pad[:, :H] = z2a
    z8_pad = z_pad.astype(F8)

    # sliding kv patches from padded z2 (original fp32 values, cast per-row)
    sw = np.lib.stride_tricks.sliding_window_view(z_pad, (KH, KW), axis=(1, 2))

    q64 = q.astype(np.float64)
    ij_off = (np.arange(KH)[:, None] * W + np.arange(KW)[None, :]).reshape(-1)  # [40]

    idc = np.zeros((128, 129), dtype=np.float32)
    idc[:, 0:128] = np.eye(128, dtype=np.float32)
    idc[:, 128] = LN64

    in_maps = []
    corrs = []
    for core in range(NCORES):
        h0 = HPC * core
        slab8 = z8_pad[:, h0 : h0 + ZROWS, :].reshape(KC, ZLEN)
        zz = np.zeros((KC, 2, ZLEN), dtype=F8)
        zz[:, 0, :] = slab8
        zz[:, 1, 0 : ZLEN - W] = slab8[:, W:]
        # kv rows indexed by flat position p = h_local*64 + w
        kvp = np.zeros((PKC, D), dtype=F8)
        hh = np.arange(PKC) // W
        ww = np.arange(PKC) % W
        real = (ww < WK) & (h0 + hh < HK)
        ridx = np.nonzero(real)[0]
        kvp[ridx] = (
            sw[:, h0 + hh[ridx], ww[ridx]].transpose(1, 0, 2, 3).reshape(-1, D)
        ).astype(F8)
        kv8 = np.ascontiguousarray(kvp.reshape(6, 128, D).transpose(1, 0, 2))
        in_maps.append(
            {"zz": zz, "qm0": qm0, "qm1": qm1, "kv8": kv8, "idc": idc}
        )
        # denominator correction: computed-but-invalid stream positions,
        # recomputed exactly (fp64) from the original values.
        bad = np.nonzero(~real)[0]
        zf = z_pad[:, h0 : h0 + ZROWS, :].reshape(KC, ZLEN).astype(np.float64)
        win = zf[:, bad[:, None] + ij_off[None, :]]  # [128, nb, 40]
        patches = win.transpose(1, 0, 2).reshape(len(bad), D)
        s_bad = q64 @ patches.T  # [160, nb]
        corrs.append(np.exp(s_bad * SCALE).sum(axis=1))

    corr = np.sum(corrs, axis=0)
    swr = sw[:, :HK, :WK]
    colsum = swr.astype(np.float64).sum(axis=(1, 2)).reshape(D)  # [5120]
    return in_maps, corr, colsum


def kernel(z1_hat, z2):
    from concourse.bass_utils import run_bass_kernel_spmd

    in_maps, corr, colsum = _host_prep(z1_hat, z2)
    if "nc" not in _CACHE:
        _CACHE["nc"] = _build_nc()
    nc = _CACHE["nc"]
    res = run_bass_kernel_spmd(nc, in_maps, list(range(NCORES)))
    num = np.broadcast_to(colsum, (PQ, D)).astype(np.float64).copy()
    den = -corr
    for r in res.results:
        ohi = r["ohi"].astype(np.float64)  # [128, 5120] = 64 * partial m0
        olo = r["olo"].astype(np.float64).reshape(96, 4, 512)
        part = np.zeros((PQ, D))
        part[0:128] = ohi
        for nt in range(10):
            rr, g = nt // 3, nt % 3
            part[128:160, nt * 512 : (nt + 1) * 512] = olo[32 * g : 32 * g + 32, rr]
        num += part / 64.0
        dv = r["den"].astype(np.float64)[:, 0] / 64.0
        den = den + np.concatenate(
            [dv[0:128], dv[128:160] + dv[160:192] + dv[192:224]]
        )
    out = (num / den[:, None]).astype(np.float32)
    # fold patches back: [160, 5120] -> [1, 128, 100, 64]
    out = out.reshape(NH, NW, KC, KH, KW).transpose(2, 0, 3, 1, 4)
    return np.ascontiguousarray(out.reshape(1, KC, H, W))
